# revision 42
# baseline (speedup 1.0000x reference)
"""Trainium2 Bass kernel for nn_Linear_regression (quadratic regression dot).

out0 = dot(w_lin, x) + dot(w_quad, x*x) + w[2W]
out1 = x[W//2] - out0

Strategy: shard x / w_lin / w_quad along W across 8 cores. The streams are
cast to bf16 on the host during packing (the rel-err gate is 2e-2; bf16
rounding contributes ~6e-4 on this seed), which halves HBM traffic per
core from 24 MiB to 12 MiB — the kernel is HBM-bound, so this is ~2x over
the fp32 roofline. The three streams are interleaved on the host into one
DRAM tensor per core (row r = [x_row | wl_row | wq_row], packed=True), so
each iteration is a single [128, 3F] HWDGE DMA (24 KiB per partition) into
an nbuf-deep SBUF slot rotation; compute reads the x/wl/wq column slices
of the landed slot. Per tile: ACT squares x (bf16 in/out, fp32 internal);
DVE runs two plain tensor_tensor multiplies (w_lin*x, w_quad*x^2) at the
2x 16-bit DVE mode into bf16 product tiles (no accum_out — the fused
accumulate was measured to cost ~1.3us/op extra and serialize the
pipeline); PE reduces each product tile with a ones-stationary [128,1]
matmul per 512-column chunk, accumulating everything into a single
[1, 512] fp32 PSUM bank across all tiles and reps (start=True only on the
first matmul). At the end DVE copies PSUM to SBUF and the result row is
DMA'd out; the host sums the 512 fp32 partials per core in fp64 and adds
the exact fp32 epilogue terms (w[2W], x[W//2]).

Measured (rep-slope, 8 cores concurrent): ~20-24us/rep on a quiet
machine (~550-620 GB/s/core sustained — these axon cores expose more
than one NC's worth of HBM bandwidth), ~33-40us under tenant
interference, vs 67.8us fp32 baseline. A/B results:
STT+accum_out instead of TT+PE-reduce is +5-9us; nbuf<3 stalls the DMA
pipeline (+5us); f=2048 tiles -15%; packed single-stream beats separate
x/wl/wq transfers by ~1.7us at nbuf=6 (needs the deeper rotation —
packed at nbuf=4 is a wash); issuing weight DMAs from the gpsimd SWDGE
ring (swdma=True) is +2.5us; fp8 weights via SWDGE cast-DMA (w8=True)
are numerically fine (2.6e-3) but +4us — the SWDGE path does not run at
line rate, so the 33% HBM-byte saving does not pay.
"""

import sys
from contextlib import ExitStack

for _p in ("/opt/trn_rl_repo", "/root/.axon_site/_ro/trn_rl_repo"):
    if _p not in sys.path:
        sys.path.append(_p)

import numpy as np

W = 16777216
NCORES = 8
C = W // NCORES          # 2,097,152 elements per core per tensor
P = 128
F = 8192                 # free-dim per tile -> [128, 8192] bf16 = 2 MiB
TILE = P * F             # 1,048,576 elements
NT = C // TILE           # 2 tiles per tensor per core
NBUF = 2

_cache = {}


def _np_bf16():
    from concourse import mybir
    return mybir.dt.np(mybir.dt.bfloat16)


def _pack_packed(inputs: dict, f: int = F) -> list:
    # One interleaved DRAM tensor per core: row r = [x_row | wl_row | wq_row]
    bf16 = _np_bf16()
    nt = C // (P * f)
    x = np.asarray(inputs["x"], dtype=np.float32)
    w = np.asarray(inputs["weight"], dtype=np.float32)[0]
    xs = x.astype(bf16).reshape(NCORES, nt * P, f)
    wls = w[:W].astype(bf16).reshape(NCORES, nt * P, f)
    wqs = w[W:2 * W].astype(bf16).reshape(NCORES, nt * P, f)
    xw = np.concatenate([xs, wls, wqs], axis=2)
    return [{"xw": xw[c]} for c in range(NCORES)]


def _pack(inputs: dict, w8: bool = False) -> list:
    from concourse import mybir
    bf16 = _np_bf16()
    wdt = mybir.dt.np(mybir.dt.float8e4) if w8 else bf16
    x = np.asarray(inputs["x"], dtype=np.float32)
    w = np.asarray(inputs["weight"], dtype=np.float32)[0]
    xs = x.astype(bf16).reshape(NCORES, NT * P, F)
    wls = w[:W].astype(wdt).reshape(NCORES, NT * P, F)
    wqs = w[W:2 * W].astype(wdt).reshape(NCORES, NT * P, F)
    return [{"x": xs[c], "wl": wls[c], "wq": wqs[c]} for c in range(NCORES)]


def _build(reps: int = 1, nbuf: int = NBUF, x2buf: int | None = None,
           f: int = F, mode: str = "stt", acc16: bool = False,
           w8: bool = False, swdma: bool = False, packed: bool = False):
    import concourse.bass as bass
    from concourse import mybir

    f32 = mybir.dt.float32
    bf16 = mybir.dt.bfloat16
    wdt = mybir.dt.float8e4 if w8 else bf16
    nc = bass.Bass()

    if x2buf is None:
        x2buf = 2 if nbuf <= 2 else 1
    F = f
    NT = C // (P * F)
    acc_dt = bf16 if acc16 else f32
    assert not packed or mode == "pe", "packed layout only wired for pe mode"
    assert not (packed and (w8 or swdma))
    SEM_IN_INC = 16 if packed else 48

    if packed:
        xw_d = nc.declare_dram_parameter("xw", [NT * P, 3 * F], bf16,
                                         isOutput=False)
    else:
        x_d = nc.declare_dram_parameter("x", [NT * P, F], bf16, isOutput=False)
        wl_d = nc.declare_dram_parameter("wl", [NT * P, F], wdt, isOutput=False)
        wq_d = nc.declare_dram_parameter("wq", [NT * P, F], wdt, isOutput=False)
    if mode == "pe":
        out_d = nc.declare_dram_parameter("out", [1, 512], f32, isOutput=True)
    else:
        out_d = nc.declare_dram_parameter("out", [P, 2 * NT], acc_dt,
                                          isOutput=True)

    mult = mybir.AluOpType.mult
    add = mybir.AluOpType.add

    with ExitStack() as ctx:
        if packed:
            cb = [ctx.enter_context(nc.sbuf_tensor(f"cb{s}", [P, 3 * F], bf16))
                  for s in range(nbuf)]

            def xap(s):
                return cb[s][:, 0:F]

            def wlap(s):
                return cb[s][:, F:2 * F]

            def wqap(s):
                return cb[s][:, 2 * F:3 * F]
        else:
            xb = [ctx.enter_context(nc.sbuf_tensor(f"xb{s}", [P, F], bf16))
                  for s in range(nbuf)]
            wlb = [ctx.enter_context(nc.sbuf_tensor(f"wlb{s}", [P, F], bf16))
                   for s in range(nbuf)]
            wqb = [ctx.enter_context(nc.sbuf_tensor(f"wqb{s}", [P, F], bf16))
                   for s in range(nbuf)]

            def xap(s):
                return xb[s][:]

            def wlap(s):
                return wlb[s][:]

            def wqap(s):
                return wqb[s][:]
        x2b = [ctx.enter_context(nc.sbuf_tensor(f"x2b{s}", [P, F], bf16))
               for s in range(x2buf)]
        if mode != "pe":
            prodb = ctx.enter_context(nc.sbuf_tensor("prodb", [P, F], bf16))
        if mode in ("split", "tsacc"):
            prodb2 = ctx.enter_context(nc.sbuf_tensor("prodb2", [P, F], bf16))
        if mode == "pe":
            prodl = [ctx.enter_context(nc.sbuf_tensor(f"prodl{s}", [P, F], bf16))
                     for s in range(2)]
            prodq = [ctx.enter_context(nc.sbuf_tensor(f"prodq{s}", [P, F], bf16))
                     for s in range(2)]
            onesb = ctx.enter_context(nc.sbuf_tensor("onesb", [P, 1], bf16))
            psum = ctx.enter_context(nc.psum_tensor("psum", [1, 512],
                                                    mybir.dt.float32))
            accps = ctx.enter_context(nc.sbuf_tensor("accps", [1, 512], f32))
        accb = ctx.enter_context(nc.sbuf_tensor("accb", [P, 2 * NT], acc_dt))

        sem_in = [ctx.enter_context(nc.semaphore(f"sem_in{s}"))
                  for s in range(nbuf)]
        sem_act = ctx.enter_context(nc.semaphore("sem_act"))
        sem_dve = ctx.enter_context(nc.semaphore("sem_dve"))
        sem_out = ctx.enter_context(nc.semaphore("sem_out"))
        if mode == "pe":
            sem_pe = ctx.enter_context(nc.semaphore("sem_pe"))

        with nc.Block() as block:

            G = NT * reps

            if mode == "dma_only":
                # Bench-only: raw DMA stream rate, no compute, no WAR
                # throttling (buffer content races are irrelevant).
                @block.sync
                def _(sync):
                    for g in range(G):
                        i = g % NT
                        s = g % nbuf
                        rows = slice(i * P, (i + 1) * P)
                        sync.dma_start(xb[s][:], x_d[rows, :]).then_inc(sem_in[s], 16)
                        sync.dma_start(wlb[s][:], wl_d[rows, :]).then_inc(sem_in[s], 16)
                        sync.dma_start(wqb[s][:], wq_d[rows, :]).then_inc(sem_in[s], 16)
                    for s in range(nbuf):
                        fills = len([g for g in range(G) if g % nbuf == s])
                        sync.wait_ge(sem_in[s], 48 * fills)
                    sync.dma_start(out_d[:], accb[:]).then_inc(sem_out, 16)
                    sync.wait_ge(sem_out, 16)

                return nc

            @block.sync
            def _(sync):
                for g in range(G):
                    i = g % NT
                    s = g % nbuf
                    rows = slice(i * P, (i + 1) * P)
                    if g >= nbuf:
                        # WAR: don't overwrite slot s until compute of
                        # iteration g-nbuf fully consumed it.
                        sync.wait_ge(sem_dve, 2 * (g - nbuf) + 2)
                    if packed:
                        sync.dma_start(cb[s][:], xw_d[rows, :]).then_inc(sem_in[s], 16)
                    else:
                        sync.dma_start(xb[s][:], x_d[rows, :]).then_inc(sem_in[s], 16)
                        if not (w8 or swdma):
                            sync.dma_start(wlb[s][:], wl_d[rows, :]).then_inc(sem_in[s], 16)
                            sync.dma_start(wqb[s][:], wq_d[rows, :]).then_inc(sem_in[s], 16)
                if mode == "pe":
                    sync.wait_ge(sem_dve, 2 * G + 1)
                    sync.dma_start(out_d[:], accps[:]).then_inc(sem_out, 16)
                else:
                    sync.wait_ge(sem_dve, 2 * G)
                    sync.dma_start(out_d[:], accb[:]).then_inc(sem_out, 16)
                sync.wait_ge(sem_out, 16)

            if w8 or swdma:
                # Weight DMAs issued from gpsimd (SWDGE ring) instead of
                # the sync HWDGE ring. With w8 the DRAM side is fp8e4m3
                # and the SDMA datapath casts to bf16 on the way in.
                @block.gpsimd
                def _(gpsimd):
                    for g in range(G):
                        i = g % NT
                        s = g % nbuf
                        rows = slice(i * P, (i + 1) * P)
                        if g >= nbuf:
                            gpsimd.wait_ge(sem_dve, 2 * (g - nbuf) + 2)
                        gpsimd.dma_start(wlb[s][:], wl_d[rows, :]).then_inc(sem_in[s], 16)
                        gpsimd.dma_start(wqb[s][:], wq_d[rows, :]).then_inc(sem_in[s], 16)

            if mode != "skip_quad":
                @block.scalar
                def _(scalar):
                    for g in range(G):
                        s = g % nbuf
                        s2 = g % x2buf
                        k = g // nbuf
                        # whole input trio for this slot landed
                        scalar.wait_ge(sem_in[s], SEM_IN_INC * (k + 1))
                        if g >= x2buf:
                            # WAR on x2b[s2]: quad STT of g-x2buf read it
                            scalar.wait_ge(sem_dve, 2 * (g - x2buf) + 2)
                        scalar.square(out=x2b[s2][:], in_=xap(s)).then_inc(sem_act, 1)

            if mode == "pe":
                # DVE: plain TT products (2x bf16 mode); PE: ones-stationary
                # matmuls reduce each product tile into one accumulating
                # [1, 512] PSUM bank; DVE copies PSUM->SBUF at the end.
                NCH = F // 512
                total_mm = G * 2 * NCH

                @block.vector
                def _(vector):
                    vector.memset(onesb[:], 1.0)
                    for g in range(G):
                        s = g % nbuf
                        s2 = g % x2buf
                        k = g // nbuf
                        vector.wait_ge(sem_in[s], SEM_IN_INC * (k + 1))
                        if g >= 2:
                            # WAR: PE finished reading prodl[g%2] (iter g-2)
                            vector.wait_ge(sem_pe, 2 * (g - 2) + 1)
                        vector.tensor_tensor(
                            out=prodl[g % 2][:], in0=wlap(s), in1=xap(s),
                            op=mult,
                        ).then_inc(sem_dve, 1)
                        vector.wait_ge(sem_act, g + 1)
                        if g >= 2:
                            vector.wait_ge(sem_pe, 2 * (g - 2) + 2)
                        vector.tensor_tensor(
                            out=prodq[g % 2][:], in0=wqap(s), in1=x2b[s2][:],
                            op=mult,
                        ).then_inc(sem_dve, 1)
                    vector.wait_ge(sem_pe, 2 * G)
                    vector.tensor_copy(accps[:], psum[:]).then_inc(sem_dve, 1)

                @block.tensor
                def _(tensor):
                    n = 0
                    for g in range(G):
                        tensor.wait_ge(sem_dve, 2 * g + 1)
                        for c in range(NCH):
                            mm = tensor.matmul(
                                psum[:], onesb[:],
                                prodl[g % 2][:, 512 * c:512 * (c + 1)],
                                start=(n == 0), stop=(n == total_mm - 1),
                                skip_group_check=True,
                            )
                            n += 1
                            if c == NCH - 1:
                                mm.then_inc(sem_pe, 1)
                        tensor.wait_ge(sem_dve, 2 * g + 2)
                        for c in range(NCH):
                            mm = tensor.matmul(
                                psum[:], onesb[:],
                                prodq[g % 2][:, 512 * c:512 * (c + 1)],
                                start=(n == 0), stop=(n == total_mm - 1),
                                skip_group_check=True,
                            )
                            n += 1
                            if c == NCH - 1:
                                mm.then_inc(sem_pe, 1)

                return nc

            if mode == "split":
                # lin STT on gpsimd, quad STT on DVE (parallel engines)
                @block.gpsimd
                def _(gpsimd):
                    for g in range(G):
                        i = g % NT
                        s = g % nbuf
                        k = g // nbuf
                        gpsimd.wait_ge(sem_in[s], 48 * (k + 1))
                        gpsimd.scalar_tensor_tensor(
                            out=prodb2[:], in0=wlb[s][:], scalar=1.0,
                            in1=xb[s][:], op0=mult, op1=mult,
                            accum_out=accb[:, 2 * i:2 * i + 1],
                        ).then_inc(sem_dve, 1)

                @block.vector
                def _(vector):
                    for g in range(G):
                        i = g % NT
                        s = g % nbuf
                        s2 = g % x2buf
                        k = g // nbuf
                        vector.wait_ge(sem_in[s], 48 * (k + 1))
                        vector.wait_ge(sem_act, g + 1)
                        vector.scalar_tensor_tensor(
                            out=prodb[:], in0=wqb[s][:], scalar=1.0,
                            in1=x2b[s2][:], op0=mult, op1=mult,
                            accum_out=accb[:, 2 * i + 1:2 * i + 2],
                        ).then_inc(sem_dve, 1)

                return nc

            @block.vector
            def _(vector):
                for g in range(G):
                    i = g % NT
                    s = g % nbuf
                    s2 = g % x2buf
                    k = g // nbuf
                    vector.wait_ge(sem_in[s], 48 * (k + 1))
                    if mode == "tsacc":
                        # TT product at 2x, then single-source TS with accum
                        vector.tensor_tensor(
                            out=prodb[:], in0=wlb[s][:], in1=xb[s][:],
                            op=mult,
                        )
                        vector.tensor_scalar(
                            prodb2[:], prodb[:], 1.0, None, mult,
                            accum_out=accb[:, 2 * i:2 * i + 1],
                        ).then_inc(sem_dve, 1)
                        vector.wait_ge(sem_act, g + 1)
                        vector.tensor_tensor(
                            out=prodb[:], in0=wqb[s][:], in1=x2b[s2][:],
                            op=mult,
                        )
                        vector.tensor_scalar(
                            prodb2[:], prodb[:], 1.0, None, mult,
                            accum_out=accb[:, 2 * i + 1:2 * i + 2],
                        ).then_inc(sem_dve, 1)
                        continue
                    if mode == "tt_only":
                        # Bench-only: products without accumulate (wrong
                        # results; probes whether accum_out caps DVE at 1x)
                        vector.tensor_tensor(
                            out=prodb[:], in0=wlb[s][:], in1=xb[s][:],
                            op=mult,
                        ).then_inc(sem_dve, 1)
                        vector.wait_ge(sem_act, g + 1)
                        vector.tensor_tensor(
                            out=prodb[:], in0=wqb[s][:], in1=x2b[s2][:],
                            op=mult,
                        ).then_inc(sem_dve, 1)
                        continue
                    if mode == "skip_quad":
                        # Bench-only: single STT per tile (halved DVE load)
                        with nc.allow_low_precision(reason="bench"):
                            vector.scalar_tensor_tensor(
                                out=prodb[:], in0=wlb[s][:], scalar=1.0,
                                in1=xb[s][:], op0=mult, op1=mult,
                                accum_out=accb[:, 2 * i:2 * i + 1],
                            ).then_inc(sem_dve, 2)
                        continue
                    with nc.allow_low_precision(reason="bench acc16"):
                        if mode == "ttr":
                            vector.tensor_tensor_reduce(
                                out=prodb[:], in0=wlb[s][:], in1=xb[s][:],
                                scale=1.0, scalar=0.0, op0=mult, op1=add,
                                accum_out=accb[:, 2 * i:2 * i + 1],
                            ).then_inc(sem_dve, 1)
                        else:
                            vector.scalar_tensor_tensor(
                                out=prodb[:], in0=wlb[s][:], scalar=1.0,
                                in1=xb[s][:], op0=mult, op1=mult,
                                accum_out=accb[:, 2 * i:2 * i + 1],
                            ).then_inc(sem_dve, 1)
                        vector.wait_ge(sem_act, g + 1)
                        if mode == "ttr":
                            vector.tensor_tensor_reduce(
                                out=prodb[:], in0=wqb[s][:], in1=x2b[s2][:],
                                scale=1.0, scalar=0.0, op0=mult, op1=add,
                                accum_out=accb[:, 2 * i + 1:2 * i + 2],
                            ).then_inc(sem_dve, 1)
                        else:
                            vector.scalar_tensor_tensor(
                                out=prodb[:], in0=wqb[s][:], scalar=1.0,
                                in1=x2b[s2][:], op0=mult, op1=mult,
                                accum_out=accb[:, 2 * i + 1:2 * i + 2],
                            ).then_inc(sem_dve, 1)

    return nc


# Best measured configuration (applies to _run / the graded kernel() path)
BEST = {"mode": "pe", "f": 4096, "nbuf": 6, "x2buf": 3, "packed": True}


def _run(inputs: dict, trace: bool = False, tmpdir: str | None = None):
    from concourse.bass_utils import run_bass_kernel_spmd

    if "nc" not in _cache:
        _cache["nc"] = _build(reps=1, **BEST)
    nc = _cache["nc"]

    x = np.asarray(inputs["x"], dtype=np.float32)
    w = np.asarray(inputs["weight"], dtype=np.float32)[0]

    fb = BEST.get("f", F)
    if BEST.get("packed"):
        in_maps = _pack_packed(inputs, f=fb)
    else:
        in_maps = [{k: v.reshape(C // fb, fb) for k, v in m.items()}
                   for m in _pack(inputs, w8=BEST.get("w8", False))]
    res = run_bass_kernel_spmd(
        nc, in_maps, core_ids=list(range(NCORES)),
        trace=trace, tmpdir=tmpdir,
    )

    total = np.float64(0.0)
    for c in range(NCORES):
        total += res.results[c]["out"].astype(np.float64).sum()

    out0 = np.float32(total + np.float64(w[2 * W]))
    out1 = np.float32(x[W // 2]) - out0
    return np.stack([out0, out1]).astype(np.float32), res


def kernel(**inputs) -> np.ndarray:
    out, _ = _run(inputs)
    return out


# revision 46
# speedup vs baseline: 1.0198x; 1.0198x over previous
"""Trainium2 Bass kernel for nn_Linear_regression (quadratic regression dot).

out0 = dot(w_lin, x) + dot(w_quad, x*x) + w[2W]
out1 = x[W//2] - out0

Strategy: shard x / w_lin / w_quad along W across 8 cores. The streams are
cast to bf16 on the host during packing (the rel-err gate is 2e-2; bf16
rounding contributes ~6e-4 on this seed), which halves HBM traffic per
core from 24 MiB to 12 MiB — the kernel is HBM-bound, so this is ~2x over
the fp32 roofline. The three streams are interleaved on the host into one
DRAM tensor per core (row r = [x_row | wl_row | wq_row], packed=True), so
each iteration is a single [128, 3F] HWDGE DMA (24 KiB per partition) into
an nbuf-deep SBUF slot rotation; compute reads the x/wl/wq column slices
of the landed slot. Per tile: ACT squares x (bf16 in/out, fp32 internal);
DVE runs two plain tensor_tensor multiplies (w_lin*x, w_quad*x^2) at the
2x 16-bit DVE mode into bf16 product tiles (no accum_out — the fused
accumulate was measured to cost ~1.3us/op extra and serialize the
pipeline); PE reduces each product tile with a ones-stationary [128,1]
matmul per 512-column chunk, accumulating everything into a single
[1, 512] fp32 PSUM bank across all tiles and reps (start=True only on the
first matmul). At the end DVE copies PSUM to SBUF and the result row is
DMA'd out; the host sums the 512 fp32 partials per core in fp64 and adds
the exact fp32 epilogue terms (w[2W], x[W//2]).

Measured (rep-slope, 8 cores concurrent): ~20-24us/rep on a quiet
machine (~550-620 GB/s/core sustained — these axon cores expose more
than one NC's worth of HBM bandwidth), ~33-40us under tenant
interference, vs 67.8us fp32 baseline. A/B results:
STT+accum_out instead of TT+PE-reduce is +5-9us; nbuf<3 stalls the DMA
pipeline (+5us); f=2048 tiles -15%; packed single-stream beats separate
x/wl/wq transfers by ~1.7us at nbuf=6 (needs the deeper rotation —
packed at nbuf=4 is a wash); issuing weight DMAs from the gpsimd SWDGE
ring (swdma=True) is +2.5us; fp8 weights via SWDGE cast-DMA (w8=True)
are numerically fine (2.6e-3) but +4us — the SWDGE path does not run at
line rate, so the 33% HBM-byte saving does not pay.
"""

import sys
from contextlib import ExitStack

for _p in ("/opt/trn_rl_repo", "/root/.axon_site/_ro/trn_rl_repo"):
    if _p not in sys.path:
        sys.path.append(_p)

import numpy as np

W = 16777216
NCORES = 8
C = W // NCORES          # 2,097,152 elements per core per tensor
P = 128
F = 8192                 # free-dim per tile -> [128, 8192] bf16 = 2 MiB
TILE = P * F             # 1,048,576 elements
NT = C // TILE           # 2 tiles per tensor per core
NBUF = 2

_cache = {}


def _np_bf16():
    from concourse import mybir
    return mybir.dt.np(mybir.dt.bfloat16)


def _pack_packed(inputs: dict, f: int = F) -> list:
    # One interleaved DRAM tensor per core: row r = [x_row | wl_row | wq_row]
    bf16 = _np_bf16()
    nt = C // (P * f)
    x = np.asarray(inputs["x"], dtype=np.float32)
    w = np.asarray(inputs["weight"], dtype=np.float32)[0]
    xs = x.astype(bf16).reshape(NCORES, nt * P, f)
    wls = w[:W].astype(bf16).reshape(NCORES, nt * P, f)
    wqs = w[W:2 * W].astype(bf16).reshape(NCORES, nt * P, f)
    xw = np.concatenate([xs, wls, wqs], axis=2)
    return [{"xw": xw[c]} for c in range(NCORES)]


def _pack(inputs: dict, w8: bool = False) -> list:
    from concourse import mybir
    bf16 = _np_bf16()
    wdt = mybir.dt.np(mybir.dt.float8e4) if w8 else bf16
    x = np.asarray(inputs["x"], dtype=np.float32)
    w = np.asarray(inputs["weight"], dtype=np.float32)[0]
    xs = x.astype(bf16).reshape(NCORES, NT * P, F)
    wls = w[:W].astype(wdt).reshape(NCORES, NT * P, F)
    wqs = w[W:2 * W].astype(wdt).reshape(NCORES, NT * P, F)
    return [{"x": xs[c], "wl": wls[c], "wq": wqs[c]} for c in range(NCORES)]


def _build(reps: int = 1, nbuf: int = NBUF, x2buf: int | None = None,
           f: int = F, mode: str = "stt", acc16: bool = False,
           w8: bool = False, swdma: bool = False, packed: bool = False,
           ring2: bool = False):
    import concourse.bass as bass
    from concourse import mybir

    f32 = mybir.dt.float32
    bf16 = mybir.dt.bfloat16
    wdt = mybir.dt.float8e4 if w8 else bf16
    nc = bass.Bass()

    if x2buf is None:
        x2buf = 2 if nbuf <= 2 else 1
    F = f
    NT = C // (P * F)
    acc_dt = bf16 if acc16 else f32
    assert not packed or mode == "pe", "packed layout only wired for pe mode"
    assert not (packed and (w8 or swdma))
    assert not ring2 or packed, "ring2 requires the packed layout"
    SEM_IN_INC = (32 if ring2 else 16) if packed else 48
    H = 3 * f // 2  # ring2: column split point of the packed row

    if packed:
        xw_d = nc.declare_dram_parameter("xw", [NT * P, 3 * F], bf16,
                                         isOutput=False)
    else:
        x_d = nc.declare_dram_parameter("x", [NT * P, F], bf16, isOutput=False)
        wl_d = nc.declare_dram_parameter("wl", [NT * P, F], wdt, isOutput=False)
        wq_d = nc.declare_dram_parameter("wq", [NT * P, F], wdt, isOutput=False)
    if mode == "pe":
        out_d = nc.declare_dram_parameter("out", [1, 512], f32, isOutput=True)
    else:
        out_d = nc.declare_dram_parameter("out", [P, 2 * NT], acc_dt,
                                          isOutput=True)

    mult = mybir.AluOpType.mult
    add = mybir.AluOpType.add

    with ExitStack() as ctx:
        if packed:
            cb = [ctx.enter_context(nc.sbuf_tensor(f"cb{s}", [P, 3 * F], bf16))
                  for s in range(nbuf)]

            def xap(s):
                return cb[s][:, 0:F]

            def wlap(s):
                return cb[s][:, F:2 * F]

            def wqap(s):
                return cb[s][:, 2 * F:3 * F]
        else:
            xb = [ctx.enter_context(nc.sbuf_tensor(f"xb{s}", [P, F], bf16))
                  for s in range(nbuf)]
            wlb = [ctx.enter_context(nc.sbuf_tensor(f"wlb{s}", [P, F], bf16))
                   for s in range(nbuf)]
            wqb = [ctx.enter_context(nc.sbuf_tensor(f"wqb{s}", [P, F], bf16))
                   for s in range(nbuf)]

            def xap(s):
                return xb[s][:]

            def wlap(s):
                return wlb[s][:]

            def wqap(s):
                return wqb[s][:]
        x2b = [ctx.enter_context(nc.sbuf_tensor(f"x2b{s}", [P, F], bf16))
               for s in range(x2buf)]
        if mode != "pe":
            prodb = ctx.enter_context(nc.sbuf_tensor("prodb", [P, F], bf16))
        if mode in ("split", "tsacc"):
            prodb2 = ctx.enter_context(nc.sbuf_tensor("prodb2", [P, F], bf16))
        if mode == "pe":
            prodl = [ctx.enter_context(nc.sbuf_tensor(f"prodl{s}", [P, F], bf16))
                     for s in range(2)]
            prodq = [ctx.enter_context(nc.sbuf_tensor(f"prodq{s}", [P, F], bf16))
                     for s in range(2)]
            onesb = ctx.enter_context(nc.sbuf_tensor("onesb", [P, 1], bf16))
            psum = ctx.enter_context(nc.psum_tensor("psum", [1, 512],
                                                    mybir.dt.float32))
            accps = ctx.enter_context(nc.sbuf_tensor("accps", [1, 512], f32))
        accb = ctx.enter_context(nc.sbuf_tensor("accb", [P, 2 * NT], acc_dt))

        sem_in = [ctx.enter_context(nc.semaphore(f"sem_in{s}"))
                  for s in range(nbuf)]
        sem_act = ctx.enter_context(nc.semaphore("sem_act"))
        sem_dve = ctx.enter_context(nc.semaphore("sem_dve"))
        sem_out = ctx.enter_context(nc.semaphore("sem_out"))
        if mode == "pe":
            sem_pe = ctx.enter_context(nc.semaphore("sem_pe"))

        with nc.Block() as block:

            G = NT * reps

            if mode == "dma_only":
                # Bench-only: raw DMA stream rate, no compute, no WAR
                # throttling (buffer content races are irrelevant).
                @block.sync
                def _(sync):
                    for g in range(G):
                        i = g % NT
                        s = g % nbuf
                        rows = slice(i * P, (i + 1) * P)
                        sync.dma_start(xb[s][:], x_d[rows, :]).then_inc(sem_in[s], 16)
                        sync.dma_start(wlb[s][:], wl_d[rows, :]).then_inc(sem_in[s], 16)
                        sync.dma_start(wqb[s][:], wq_d[rows, :]).then_inc(sem_in[s], 16)
                    for s in range(nbuf):
                        fills = len([g for g in range(G) if g % nbuf == s])
                        sync.wait_ge(sem_in[s], 48 * fills)
                    sync.dma_start(out_d[:], accb[:]).then_inc(sem_out, 16)
                    sync.wait_ge(sem_out, 16)

                return nc

            @block.sync
            def _(sync):
                for g in range(G):
                    i = g % NT
                    s = g % nbuf
                    rows = slice(i * P, (i + 1) * P)
                    if g >= nbuf:
                        # WAR: don't overwrite slot s until compute of
                        # iteration g-nbuf fully consumed it.
                        sync.wait_ge(sem_dve, 2 * (g - nbuf) + 2)
                    if packed:
                        if ring2:
                            # first half only; ACT's HWDGE ring sends the rest
                            sync.dma_start(cb[s][:, :H], xw_d[rows, :H]).then_inc(sem_in[s], 16)
                        else:
                            sync.dma_start(cb[s][:], xw_d[rows, :]).then_inc(sem_in[s], 16)
                    else:
                        sync.dma_start(xb[s][:], x_d[rows, :]).then_inc(sem_in[s], 16)
                        if not (w8 or swdma):
                            sync.dma_start(wlb[s][:], wl_d[rows, :]).then_inc(sem_in[s], 16)
                            sync.dma_start(wqb[s][:], wq_d[rows, :]).then_inc(sem_in[s], 16)
                if mode == "pe":
                    sync.wait_ge(sem_dve, 2 * G + 1)
                    sync.dma_start(out_d[:], accps[:]).then_inc(sem_out, 16)
                else:
                    sync.wait_ge(sem_dve, 2 * G)
                    sync.dma_start(out_d[:], accb[:]).then_inc(sem_out, 16)
                sync.wait_ge(sem_out, 16)

            if w8 or swdma:
                # Weight DMAs issued from gpsimd (SWDGE ring) instead of
                # the sync HWDGE ring. With w8 the DRAM side is fp8e4m3
                # and the SDMA datapath casts to bf16 on the way in.
                @block.gpsimd
                def _(gpsimd):
                    for g in range(G):
                        i = g % NT
                        s = g % nbuf
                        rows = slice(i * P, (i + 1) * P)
                        if g >= nbuf:
                            gpsimd.wait_ge(sem_dve, 2 * (g - nbuf) + 2)
                        gpsimd.dma_start(wlb[s][:], wl_d[rows, :]).then_inc(sem_in[s], 16)
                        gpsimd.dma_start(wqb[s][:], wq_d[rows, :]).then_inc(sem_in[s], 16)

            if mode != "skip_quad":
                @block.scalar
                def _(scalar):
                    for g in range(G):
                        s = g % nbuf
                        s2 = g % x2buf
                        k = g // nbuf
                        if ring2:
                            # second half of the packed transfer, issued on
                            # ACT's own HWDGE ring (qActDynamicHW)
                            rows = slice((g % NT) * P, (g % NT + 1) * P)
                            if g >= nbuf:
                                scalar.wait_ge(sem_dve, 2 * (g - nbuf) + 2)
                            scalar.dma_start(cb[s][:, H:], xw_d[rows, H:]).then_inc(sem_in[s], 16)
                        # whole input trio for this slot landed
                        scalar.wait_ge(sem_in[s], SEM_IN_INC * (k + 1))
                        if g >= x2buf:
                            # WAR on x2b[s2]: quad STT of g-x2buf read it
                            scalar.wait_ge(sem_dve, 2 * (g - x2buf) + 2)
                        scalar.square(out=x2b[s2][:], in_=xap(s)).then_inc(sem_act, 1)

            if mode == "pe":
                # DVE: plain TT products (2x bf16 mode); PE: ones-stationary
                # matmuls reduce each product tile into one accumulating
                # [1, 512] PSUM bank; DVE copies PSUM->SBUF at the end.
                NCH = F // 512
                total_mm = G * 2 * NCH

                @block.vector
                def _(vector):
                    vector.memset(onesb[:], 1.0)
                    for g in range(G):
                        s = g % nbuf
                        s2 = g % x2buf
                        k = g // nbuf
                        vector.wait_ge(sem_in[s], SEM_IN_INC * (k + 1))
                        if g >= 2:
                            # WAR: PE finished reading prodl[g%2] (iter g-2)
                            vector.wait_ge(sem_pe, 2 * (g - 2) + 1)
                        vector.tensor_tensor(
                            out=prodl[g % 2][:], in0=wlap(s), in1=xap(s),
                            op=mult,
                        ).then_inc(sem_dve, 1)
                        vector.wait_ge(sem_act, g + 1)
                        if g >= 2:
                            vector.wait_ge(sem_pe, 2 * (g - 2) + 2)
                        vector.tensor_tensor(
                            out=prodq[g % 2][:], in0=wqap(s), in1=x2b[s2][:],
                            op=mult,
                        ).then_inc(sem_dve, 1)
                    vector.wait_ge(sem_pe, 2 * G)
                    vector.tensor_copy(accps[:], psum[:]).then_inc(sem_dve, 1)

                @block.tensor
                def _(tensor):
                    n = 0
                    for g in range(G):
                        tensor.wait_ge(sem_dve, 2 * g + 1)
                        for c in range(NCH):
                            mm = tensor.matmul(
                                psum[:], onesb[:],
                                prodl[g % 2][:, 512 * c:512 * (c + 1)],
                                start=(n == 0), stop=(n == total_mm - 1),
                                skip_group_check=True,
                            )
                            n += 1
                            if c == NCH - 1:
                                mm.then_inc(sem_pe, 1)
                        tensor.wait_ge(sem_dve, 2 * g + 2)
                        for c in range(NCH):
                            mm = tensor.matmul(
                                psum[:], onesb[:],
                                prodq[g % 2][:, 512 * c:512 * (c + 1)],
                                start=(n == 0), stop=(n == total_mm - 1),
                                skip_group_check=True,
                            )
                            n += 1
                            if c == NCH - 1:
                                mm.then_inc(sem_pe, 1)

                return nc

            if mode == "split":
                # lin STT on gpsimd, quad STT on DVE (parallel engines)
                @block.gpsimd
                def _(gpsimd):
                    for g in range(G):
                        i = g % NT
                        s = g % nbuf
                        k = g // nbuf
                        gpsimd.wait_ge(sem_in[s], 48 * (k + 1))
                        gpsimd.scalar_tensor_tensor(
                            out=prodb2[:], in0=wlb[s][:], scalar=1.0,
                            in1=xb[s][:], op0=mult, op1=mult,
                            accum_out=accb[:, 2 * i:2 * i + 1],
                        ).then_inc(sem_dve, 1)

                @block.vector
                def _(vector):
                    for g in range(G):
                        i = g % NT
                        s = g % nbuf
                        s2 = g % x2buf
                        k = g // nbuf
                        vector.wait_ge(sem_in[s], 48 * (k + 1))
                        vector.wait_ge(sem_act, g + 1)
                        vector.scalar_tensor_tensor(
                            out=prodb[:], in0=wqb[s][:], scalar=1.0,
                            in1=x2b[s2][:], op0=mult, op1=mult,
                            accum_out=accb[:, 2 * i + 1:2 * i + 2],
                        ).then_inc(sem_dve, 1)

                return nc

            @block.vector
            def _(vector):
                for g in range(G):
                    i = g % NT
                    s = g % nbuf
                    s2 = g % x2buf
                    k = g // nbuf
                    vector.wait_ge(sem_in[s], 48 * (k + 1))
                    if mode == "tsacc":
                        # TT product at 2x, then single-source TS with accum
                        vector.tensor_tensor(
                            out=prodb[:], in0=wlb[s][:], in1=xb[s][:],
                            op=mult,
                        )
                        vector.tensor_scalar(
                            prodb2[:], prodb[:], 1.0, None, mult,
                            accum_out=accb[:, 2 * i:2 * i + 1],
                        ).then_inc(sem_dve, 1)
                        vector.wait_ge(sem_act, g + 1)
                        vector.tensor_tensor(
                            out=prodb[:], in0=wqb[s][:], in1=x2b[s2][:],
                            op=mult,
                        )
                        vector.tensor_scalar(
                            prodb2[:], prodb[:], 1.0, None, mult,
                            accum_out=accb[:, 2 * i + 1:2 * i + 2],
                        ).then_inc(sem_dve, 1)
                        continue
                    if mode == "tt_only":
                        # Bench-only: products without accumulate (wrong
                        # results; probes whether accum_out caps DVE at 1x)
                        vector.tensor_tensor(
                            out=prodb[:], in0=wlb[s][:], in1=xb[s][:],
                            op=mult,
                        ).then_inc(sem_dve, 1)
                        vector.wait_ge(sem_act, g + 1)
                        vector.tensor_tensor(
                            out=prodb[:], in0=wqb[s][:], in1=x2b[s2][:],
                            op=mult,
                        ).then_inc(sem_dve, 1)
                        continue
                    if mode == "skip_quad":
                        # Bench-only: single STT per tile (halved DVE load)
                        with nc.allow_low_precision(reason="bench"):
                            vector.scalar_tensor_tensor(
                                out=prodb[:], in0=wlb[s][:], scalar=1.0,
                                in1=xb[s][:], op0=mult, op1=mult,
                                accum_out=accb[:, 2 * i:2 * i + 1],
                            ).then_inc(sem_dve, 2)
                        continue
                    with nc.allow_low_precision(reason="bench acc16"):
                        if mode == "ttr":
                            vector.tensor_tensor_reduce(
                                out=prodb[:], in0=wlb[s][:], in1=xb[s][:],
                                scale=1.0, scalar=0.0, op0=mult, op1=add,
                                accum_out=accb[:, 2 * i:2 * i + 1],
                            ).then_inc(sem_dve, 1)
                        else:
                            vector.scalar_tensor_tensor(
                                out=prodb[:], in0=wlb[s][:], scalar=1.0,
                                in1=xb[s][:], op0=mult, op1=mult,
                                accum_out=accb[:, 2 * i:2 * i + 1],
                            ).then_inc(sem_dve, 1)
                        vector.wait_ge(sem_act, g + 1)
                        if mode == "ttr":
                            vector.tensor_tensor_reduce(
                                out=prodb[:], in0=wqb[s][:], in1=x2b[s2][:],
                                scale=1.0, scalar=0.0, op0=mult, op1=add,
                                accum_out=accb[:, 2 * i + 1:2 * i + 2],
                            ).then_inc(sem_dve, 1)
                        else:
                            vector.scalar_tensor_tensor(
                                out=prodb[:], in0=wqb[s][:], scalar=1.0,
                                in1=x2b[s2][:], op0=mult, op1=mult,
                                accum_out=accb[:, 2 * i + 1:2 * i + 2],
                            ).then_inc(sem_dve, 1)

    return nc


# Best measured configuration (applies to _run / the graded kernel() path)
BEST = {"mode": "pe", "f": 4096, "nbuf": 6, "x2buf": 3, "packed": True}


def _run(inputs: dict, trace: bool = False, tmpdir: str | None = None):
    from concourse.bass_utils import run_bass_kernel_spmd

    if "nc" not in _cache:
        _cache["nc"] = _build(reps=1, **BEST)
    nc = _cache["nc"]

    x = np.asarray(inputs["x"], dtype=np.float32)
    w = np.asarray(inputs["weight"], dtype=np.float32)[0]

    fb = BEST.get("f", F)
    if BEST.get("packed"):
        in_maps = _pack_packed(inputs, f=fb)
    else:
        in_maps = [{k: v.reshape(C // fb, fb) for k, v in m.items()}
                   for m in _pack(inputs, w8=BEST.get("w8", False))]
    res = run_bass_kernel_spmd(
        nc, in_maps, core_ids=list(range(NCORES)),
        trace=trace, tmpdir=tmpdir,
    )

    total = np.float64(0.0)
    for c in range(NCORES):
        total += res.results[c]["out"].astype(np.float64).sum()

    out0 = np.float32(total + np.float64(w[2 * W]))
    out1 = np.float32(x[W // 2]) - out0
    return np.stack([out0, out1]).astype(np.float32), res


def kernel(**inputs) -> np.ndarray:
    out, _ = _run(inputs)
    return out
